# revision 13
# baseline (speedup 1.0000x reference)
"""Trainium2 Bass kernel for nn_Affine_Module_Abla (VN-style affine + VNLeakyReLU).

Math (per batch b, point n, channel d<128, with u=J[...,0], v=J[...,1], X):
  b1 = u/||u||; w = v - (u.v/||u||^2) u; b2 = w/||w||; b3 = b1 x b2
  a = (b1.X, b2.X, b3.X)
  x = M3 @ a   (M3 = A+B+C);  d = (Wdir@M3) @ a = WM @ a
  out = x - 0.8*min(dot,0)/(dns+eps)*d,  dot = x.d over i, dns = d.d over i

Sharding: batch B=8 -> one batch per NeuronCore (8 cores).

Host prep per core: planes tensor [D=128, 11, N] with plane order
[u0,u1,u2,u0,u1, v0,v1,v2, x0,x1,x2] (u duplicated so the cross product
reads are contiguous rotations), weights (WM pre-scaled by 1/64 for the fp16
epilogue), plus identity matrix for PE-side reductions.

Device: stage A fp32 on DVE (stacked [128,3T] instructions), squares on ACT,
3-term reductions via identity-matmul PSUM accumulation on the PE, Y/D
matmuls in fp32, epilogue in fp16, output stored fp16->f32 via SWDGE cast.
"""

import numpy as np

_B, _N, _D, _F = 8, 4096, 128, 256
_T = 512
_T3 = 3 * _T
_NCHUNK = _N // _T
_EPS = 1e-6  # VNLeakyReLU eps
_DSC = 64.0  # d is computed scaled by 1/64 for fp16 range
_EPS_S = _EPS / (_DSC * _DSC)

_cache = {}


def _build_nc():
    import concourse.bacc as bacc
    import concourse.mybir as mybir
    from concourse.tile import TileContext

    f32 = mybir.dt.float32
    f32r = mybir.dt.float32r
    f16 = mybir.dt.float16
    Alu = mybir.AluOpType

    nc = bacc.Bacc("TRN2", target_bir_lowering=False, num_swdge_queues=2)
    UVX = nc.declare_dram_parameter("uvx", [_D, 11, _N], f32, isOutput=False)
    WY = nc.declare_dram_parameter("wy", [2, _D, 128], f32, isOutput=False)
    WD = nc.declare_dram_parameter("wd", [2, _D, 128], f32, isOutput=False)
    EYE = nc.declare_dram_parameter("eye", [_D, 128], f32, isOutput=False)
    OUT = nc.declare_dram_parameter("out", [_F, 3, _N], f32, isOutput=True)

    with TileContext(nc) as tc:
        with (
            tc.tile_pool(name="w", bufs=1) as wp,
            tc.tile_pool(name="io", bufs=2) as iop,
            tc.tile_pool(name="st", bufs=1) as stp,
            tc.tile_pool(name="ab", bufs=2) as abp,
            tc.tile_pool(name="ep", bufs=2) as epp,
            tc.tile_pool(name="ps", bufs=1, space="PSUM") as psp,
        ):
            wy, wd = [], []
            for h in range(2):
                w1 = wp.tile([_D, 128], f32, tag=f"wy{h}", name=f"wy{h}")
                nc.sync.dma_start(out=w1[:], in_=WY[h])
                w2 = wp.tile([_D, 128], f32, tag=f"wd{h}", name=f"wd{h}")
                nc.sync.dma_start(out=w2[:], in_=WD[h])
                wy.append(w1)
                wd.append(w2)
            eyef = wp.tile([_D, 128], f32, tag="eyef", name="eyef")
            nc.sync.dma_start(out=eyef[:], in_=EYE[:])
            eye = wp.tile([_D, 128], f32r, tag="eye", name="eye")
            nc.vector.tensor_copy(eye[:], eyef[:])
            eye16 = wp.tile([_D, 128], f16, tag="eye16", name="eye16")
            nc.vector.tensor_copy(eye16[:], eyef[:])

            def tt(o, a, b, op):
                nc.vector.tensor_tensor(o, a, b, op)

            for ci in range(_NCHUNK):
                sl = slice(ci * _T, (ci + 1) * _T)

                ut = iop.tile([_D, 5 * _T], f32, tag="ut", name="ut")
                nc.sync.dma_start(
                    out=ut[:].rearrange("p (o t) -> p o t", o=5),
                    in_=UVX[:, 0:5, sl],
                )
                vt = iop.tile([_D, _T3], f32, tag="vt", name="vt")
                nc.sync.dma_start(
                    out=vt[:].rearrange("p (o t) -> p o t", o=3),
                    in_=UVX[:, 5:8, sl],
                )
                xt = iop.tile([_D, _T3], f32, tag="xt", name="xt")
                nc.sync.dma_start(
                    out=xt[:].rearrange("p (o t) -> p o t", o=3),
                    in_=UVX[:, 8:11, sl],
                )
                u3 = ut[:, 0:_T3]

                def S(tag, fd=_T3, dt=f32, bufs=None, pool=stp):
                    return pool.tile([_D, fd], dt, tag=tag, name=tag, bufs=bufs)

                def pe_sum3(tag_ps, stack, ey, ptag, pbufs):
                    ps = psp.tile([128, _T], f32, tag=ptag, name=tag_ps, bufs=pbufs)
                    for j in range(3):
                        nc.tensor.matmul(
                            ps[:],
                            ey[:],
                            stack[:, j * _T : (j + 1) * _T],
                            start=(j == 0),
                            stop=(j == 2),
                        )
                    return ps

                # c = u.v ; nu2 = u.u
                cm = S("cm")
                tt(cm[:], u3, vt[:], Alu.mult)
                cuv = S("cuv", fd=_T)
                tt(cuv[:], cm[:, 0:_T], cm[:, _T : 2 * _T], Alu.add)
                tt(cuv[:], cuv[:], cm[:, 2 * _T : _T3], Alu.add)

                squ = S("squ")
                nc.scalar.square(squ[:], u3)
                nu2 = S("nu2", fd=_T)
                tt(nu2[:], squ[:, 0:_T], squ[:, _T : 2 * _T], Alu.add)
                tt(nu2[:], nu2[:], squ[:, 2 * _T : _T3], Alu.add)

                inv_u = S("inv_u", fd=_T)
                rscr = S("rscr", fd=_T)
                nc.vector.reciprocal_approx_accurate(
                    out=inv_u[:], in_=nu2[:], scratch=rscr[:]
                )
                t_ = S("t", fd=_T)
                tt(t_[:], cuv[:], inv_u[:], Alu.mult)

                # w = v - t*u  (stacked, t broadcast), then dup-pad w
                tu = S("tu")
                tt(
                    tu[:].rearrange("p (o t) -> p o t", o=3),
                    u3.rearrange("p (o t) -> p o t", o=3),
                    t_[:].rearrange("p (o t) -> p o t", o=1).broadcast_to(
                        [128, 3, _T]
                    ),
                    Alu.mult,
                )
                wt = S("wt", fd=5 * _T)
                tt(wt[:, 0:_T3], vt[:], tu[:], Alu.subtract)
                nc.scalar.copy(wt[:, _T3 : 5 * _T], wt[:, 0 : 2 * _T])
                w3 = wt[:, 0:_T3]

                # nw2 = w.w via ACT squares (f32r) + PE accumulation
                sqw = S("sqw", dt=f32r)
                nc.scalar.square(sqw[:], w3)
                nw2_ps = pe_sum3("nw2_ps", sqw, eye, "psA", 2)
                nw2 = S("nw2", fd=_T)
                nc.vector.tensor_scalar_max(nw2[:], nw2_ps[:], 1e-30)
                inv_w = S("inv_w", fd=_T)
                nc.vector.reciprocal_approx_fast(out=inv_w[:], in_=nw2[:])
                rs_u = S("rs_u", fd=_T)
                nc.scalar.sqrt(rs_u[:], inv_u[:])
                rs_w = S("rs_w", fd=_T)
                nc.scalar.sqrt(rs_w[:], inv_w[:])

                # p = u.X, wX = w.X, det = (u x w).X via PE accumulation
                pm = S("pm", dt=f32r)
                tt(pm[:], u3, xt[:], Alu.mult)
                p_ps = pe_sum3("p_ps", pm, eye, "psA", 2)

                wm = S("wm", dt=f32r)
                tt(wm[:], w3, xt[:], Alu.mult)
                w_ps = pe_sum3("w_ps", wm, eye, "psA", 2)

                cpx = S("cpx")
                tt(cpx[:], ut[:, _T : 4 * _T], wt[:, 2 * _T : 5 * _T], Alu.mult)
                cmx = S("cmx")
                tt(cmx[:], ut[:, 2 * _T : 5 * _T], wt[:, _T : 4 * _T], Alu.mult)
                crx = S("crx")
                tt(crx[:], cpx[:], cmx[:], Alu.subtract)
                dm = S("dm", dt=f32r)
                tt(dm[:], crx[:], xt[:], Alu.mult)
                d_ps = pe_sum3("d_ps", dm, eye, "psA", 2)

                a0 = abp.tile([_D, _T], f32, tag="a0", name="a0")
                tt(a0[:], p_ps[:], rs_u[:], Alu.mult)
                a1 = abp.tile([_D, _T], f32, tag="a1", name="a1")
                tt(a1[:], w_ps[:], rs_w[:], Alu.mult)
                s2c = S("s2c", fd=_T)
                tt(s2c[:], rs_u[:], rs_w[:], Alu.mult)
                a2 = abp.tile([_D, _T], f32, tag="a2", name="a2")
                tt(a2[:], d_ps[:], s2c[:], Alu.mult)
                aa = [a0, a1, a2]

                for h in range(2):
                    px = [
                        psp.tile([128, _T], f32, tag="mm", name=f"px{i}", bufs=4)
                        for i in range(3)
                    ]
                    pd = [
                        psp.tile([128, _T], f32, tag="mm", name=f"pd{i}", bufs=4)
                        for i in range(3)
                    ]
                    for i in range(3):
                        nc.tensor.matmul(
                            px[i][:], wy[h][:], aa[i][:], start=True, stop=True
                        )
                    for i in range(3):
                        nc.tensor.matmul(
                            pd[i][:], wd[h][:], aa[i][:], start=True, stop=True
                        )

                    # fp16 epilogue (d scaled by 1/64)
                    x16 = S("x16", dt=f16, pool=epp)
                    d16 = S("d16", dt=f16, pool=epp)
                    for i in range(3):
                        nc.scalar.copy(x16[:, i * _T : (i + 1) * _T], px[i][:])
                        nc.scalar.copy(d16[:, i * _T : (i + 1) * _T], pd[i][:])

                    pr = S("pr", dt=f16, pool=epp)
                    tt(pr[:], x16[:], d16[:], Alu.mult)
                    dot_ps = pe_sum3("dot_ps", pr, eye16, "psE", 2)

                    dq = S("dq", dt=f16, pool=epp)
                    nc.scalar.square(dq[:], d16[:])
                    dns_ps = pe_sum3("dns_ps", dq, eye16, "psE", 2)

                    denom = S("denom", fd=_T, pool=epp)
                    nc.vector.tensor_scalar(
                        out=denom[:],
                        in0=dns_ps[:],
                        scalar1=_EPS_S,
                        scalar2=1.25,
                        op0=Alu.add,
                        op1=Alu.mult,
                    )
                    inv = S("inv", fd=_T, pool=epp)
                    nc.vector.reciprocal_approx_fast(out=inv[:], in_=denom[:])
                    s_ = S("s", fd=_T, dt=f16, pool=epp)
                    nc.vector.scalar_tensor_tensor(
                        s_[:], dot_ps[:], 0.0, inv[:], Alu.min, Alu.mult
                    )

                    g = S("g", dt=f16, pool=epp)
                    tt(
                        g[:].rearrange("p (o t) -> p o t", o=3),
                        d16[:].rearrange("p (o t) -> p o t", o=3),
                        s_[:].rearrange("p (o t) -> p o t", o=1).broadcast_to(
                            [128, 3, _T]
                        ),
                        Alu.mult,
                    )
                    o = S("o", dt=f16, pool=epp, bufs=3)
                    tt(o[:], x16[:], g[:], Alu.subtract)
                    nc.gpsimd.dma_start(
                        out=OUT[h * 128 : (h + 1) * 128, :, sl],
                        in_=o[:].rearrange("p (o t) -> p o t", o=3),
                    )

    nc.compile()
    return nc


def _get_nc():
    if "nc" not in _cache:
        _cache["nc"] = _build_nc()
    return _cache["nc"]


def _host_prep(X, J, Amat, Bmat, Cmat, Wdir):
    X = np.ascontiguousarray(np.asarray(X, dtype=np.float32))
    J = np.ascontiguousarray(np.asarray(J, dtype=np.float32))
    Amat = np.asarray(Amat, dtype=np.float32)
    Bmat = np.asarray(Bmat, dtype=np.float32)
    Cmat = np.asarray(Cmat, dtype=np.float32)
    Wdir = np.asarray(Wdir, dtype=np.float32)

    M3 = Amat + Bmat + Cmat  # [F, D]
    WM = (Wdir @ M3) / _DSC  # [F, D], pre-scaled for the fp16 epilogue
    WY = np.ascontiguousarray(np.stack([M3[:128, :].T, M3[128:, :].T]))  # [2,D,128]
    WD = np.ascontiguousarray(np.stack([WM[:128, :].T, WM[128:, :].T]))
    EYE = np.eye(_D, dtype=np.float32)

    in_maps = []
    for b in range(_B):
        uvx = np.empty((_D, 11, _N), dtype=np.float32)
        Jt = J[b].transpose(3, 2, 1, 0)  # [2, 3, D, N]  (l, i, d, n)
        uvx[:, 0:3] = Jt[0].transpose(1, 0, 2)  # u planes as [D, 3, N]
        uvx[:, 3:5] = Jt[0][0:2].transpose(1, 0, 2)  # u0, u1 duplicated
        uvx[:, 5:8] = Jt[1].transpose(1, 0, 2)
        uvx[:, 8:11] = X[b].transpose(1, 2, 0)  # [D, 3, N]
        in_maps.append({"uvx": uvx, "wy": WY, "wd": WD, "eye": EYE})
    return in_maps


def run(X, J, Amat, Bmat, Cmat, Wdir, device=None, trace=False):
    from concourse.bass_utils import run_bass_kernel_spmd

    nc = _get_nc()
    in_maps = _host_prep(X, J, Amat, Bmat, Cmat, Wdir)
    res = run_bass_kernel_spmd(nc, in_maps, list(range(_B)), trace=trace)
    out = np.stack([res.results[b]["out"] for b in range(_B)], axis=0)
    return out, res


def kernel(X, J, Amat, Bmat, Cmat, Wdir, device=None):
    out, _ = run(X, J, Amat, Bmat, Cmat, Wdir, device)
    return out
